# revision 33
# baseline (speedup 1.0000x reference)
"""Trainium2 Bass kernel for nn_AttnGate (sparse attention block-mask).

Per (batch, k-head): Qproj pools the GQA query group into one gate query
(PE matmuls, 8x-redundant big-N form), RoPE (host-tiled cos/sin, DVE),
pooled QK block scores vs the compressed key cache (fused mul+accum
split across DVE and GPSIMD), exact top-(budget-sw) via normalized
per-row bisection (DVE), block mask assembly.

Softmax and the 1/sqrt(Dg) scale are monotonic per-row, so top-k on raw
scores selects the identical set - they are skipped.

Sharding: batch dim across 8 NeuronCores (8 batches/core), wq replicated.
k_compressed streams over both HWDGE queues (sync + scalar engines).
"""

import sys
import numpy as np

for _p in ("/opt/trn_rl_repo",):
    if _p not in sys.path:
        sys.path.insert(0, _p)

import concourse.bass as bass
import concourse.bacc as bacc
import concourse.mybir as mybir
from concourse.tile import TileContext

F32 = mybir.dt.float32
F32R = mybir.dt.float32r
U8 = mybir.dt.uint8
OP = mybir.AluOpType
AX = mybir.AxisListType

# Problem shape (hardcoded per spec)
B, HQ, HK, G, DM, DG, S = 64, 32, 8, 4, 128, 128, 512
NCORES = 8
BL = B // NCORES          # batches per core
SW = 16                   # block_sliding_window_size
BUDGET = 64               # block_budget
KEXTRA = BUDGET - SW      # 48 top-k picks
NSTOP = S - SW            # 496 eligible columns
SCH = S // 128            # 4 s-chunks of 128
N_ITER = 18               # bisection iterations
POOL_NB = 4               # batches b < POOL_NB: products on GPSIMD, else DVE
QPROJ_F32R = False        # fp32r needs pre-rounded (lossy) inputs; keep fp32


def build_nc(bl=BL, n_iter=N_ITER, pool_nb=POOL_NB, qproj_f32r=QPROJ_F32R):
    """Build the Bass program for one core handling `bl` batches.

    Output mask rows are b-major: row r = b*HK + h.
    """
    npairs = HK * bl
    nc = bacc.Bacc(trn_type="TRN2", target_bir_lowering=False)

    # ---- DRAM I/O ----
    # wqg: wq rearranged (i, (h g o)) so one h-chunk is a [128, G*DG] block.
    wqg = nc.dram_tensor("wqg", [DM, HK * G * DG], F32, kind="ExternalInput")
    # qTg: q rearranged (i, (h g b)).
    qTg = nc.dram_tensor("qTg", [DM, HK * G * bl], F32, kind="ExternalInput")
    kc = nc.dram_tensor("kc", [bl, S, HK, DG], F32, kind="ExternalInput")
    # cos8/sinR8: [bl, HK*DG] host-tiled; sinR8 has rotate-half sign folded in.
    cos8 = nc.dram_tensor("cos8", [bl, HK * DG], F32, kind="ExternalInput")
    sinR8 = nc.dram_tensor("sinR8", [bl, HK * DG], F32, kind="ExternalInput")
    eye = nc.dram_tensor("eye", [128, 128], F32, kind="ExternalInput")
    mask_u8 = nc.dram_tensor("mask_u8", [npairs, S], U8, kind="ExternalOutput")

    with TileContext(nc) as tc:
        with (
            tc.tile_pool(name="const", bufs=1) as constp,
            tc.tile_pool(name="qs", bufs=1) as qp,
            tc.tile_pool(name="qpsum", bufs=1, space="PSUM") as qpsp,
            tc.tile_pool(name="tpsum", bufs=2, space="PSUM") as tpsp,
            tc.tile_pool(name="kpool", bufs=6) as kp,
            tc.tile_pool(name="ppool", bufs=2) as pp,
            tc.tile_pool(name="bcast", bufs=3) as bcp,
            tc.tile_pool(name="sc", bufs=1) as scp,
            tc.tile_pool(name="bis", bufs=2) as bp,
            tc.tile_pool(name="dram", bufs=1, space="DRAM") as dp,
        ):
            # ---- small inputs on the scalar queue (k owns sync alone:
            # one unimpeded HWDGE queue sustains ~300 GB/s, two contend) ----
            qT_sb = constp.tile([DM, G * HK * bl], F32, tag="qT")
            nc.scalar.dma_start(qT_sb[:], qTg[:, :])
            cos_sb = constp.tile([bl, HK * DG], F32, tag="cos8")
            nc.scalar.dma_start(cos_sb[:], cos8[:, :])
            sin_sb = constp.tile([bl, HK * DG], F32, tag="sin8")
            nc.scalar.dma_start(sin_sb[:], sinR8[:, :])

            # wq in h-major chunks on the scalar queue so the per-h Qproj
            # matmuls pipeline with the wq arrival.
            wq_sb = qp.tile([DM, HK * G * DG], F32, tag="wq")
            for h in range(HK):
                nc.scalar.dma_start(
                    wq_sb[:, h * G * DG:(h + 1) * G * DG],
                    wqg[:, h * G * DG:(h + 1) * G * DG],
                )
            eye_sb = constp.tile([128, 128], F32, tag="eye")
            nc.scalar.dma_start(eye_sb[:], eye[:, :])

            # ---- k tiles: [128, (sc h d)] per batch, all on sync ----
            kts = []
            for b in range(bl):
                kt = kp.tile([128, SCH * HK * DG], F32, tag="kt", name=f"kt{b}")
                src = kc[b].rearrange("(sc p) h d -> p sc (h d)", p=128)
                nc.sync.dma_start(kt[:], src)
                kts.append(kt)

            # ---- Qproj: per (h, g) matmul, out [bl, DG] at base partition 0;
            # h-blocks packed 4-wide into two 1-bank PSUM tiles so the copies
            # out read from partition 0 (engine partition-quadrant rule).
            qp_ps_a = qpsp.tile([bl, 512], F32, tag="qpa")  # h 0-3
            qp_ps_b = qpsp.tile([bl, 512], F32, tag="qpb")  # h 4-7
            for h in range(HK):
                dst = qp_ps_a if h < 4 else qp_ps_b
                off = (h % 4) * DG
                for g in range(G):
                    hg = h * G + g
                    nc.tensor.matmul(
                        dst[0:bl, off:off + DG],
                        qT_sb[:, hg * bl:(hg + 1) * bl],
                        wq_sb[:, hg * DG:(hg + 1) * DG],
                        start=(g == 0), stop=(g == G - 1))

            # qdB [bl, (h d)] — layout matches the two PSUM tiles directly
            qdB = qp.tile([bl, HK * DG], F32, tag="qdB")
            nc.scalar.copy(qdB[0:bl, 0:512], qp_ps_a[0:bl, :])
            nc.scalar.copy(qdB[0:bl, 512:1024], qp_ps_b[0:bl, :])

            # ---- RoPE: qdN = qdB*cos8 + swap_halves(qdB)*sinR8 ----
            qrot = qp.tile([bl, HK * DG], F32, tag="qrot")
            qdB_v = qdB[:].rearrange("b (h t d) -> b h t d", h=HK, t=2)
            qrot_v = qrot[:].rearrange("b (h t d) -> b h t d", h=HK, t=2)
            nc.scalar.copy(qrot_v[:, :, 0, :], qdB_v[:, :, 1, :])
            nc.scalar.copy(qrot_v[:, :, 1, :], qdB_v[:, :, 0, :])
            t1 = qp.tile([bl, HK * DG], F32, tag="t1")
            nc.vector.tensor_mul(t1[:], qdB[:], cos_sb[:])
            qdN = qp.tile([bl, HK * DG], F32, tag="qdN")
            nc.vector.scalar_tensor_tensor(
                out=qdN[:], in0=qrot[:], scalar=0.0, in1=sin_sb[:],
                op0=OP.add, op1=OP.mult)
            nc.vector.tensor_add(qdN[:], qdN[:], t1[:])
            # qdN rows -> DRAM so per-batch partition-broadcast DMAs can
            # replicate one row across all 128 partitions.
            qdram = dp.tile([bl, HK * DG], F32, tag="qdram")
            nc.scalar.dma_start(qdram[:], qdN[:])


            # ---- scores: for each b, broadcast qdN row then fused
            # mul+accum per (sc, h); h < pool_h0 on DVE, rest on GPSIMD.
            # stall cols (sc, b, h).
            stall = scp.tile([128, SCH * bl * HK], F32, tag="stall")
            stall_v = stall[:].rearrange("p (sc b h) -> p sc b h", sc=SCH, b=bl)
            for b in range(bl):
                # broadcast qdN row b down 128 partitions (DRAM roundtrip on
                # the scalar HWDGE queue, idle after wq arrives).
                bc = bcp.tile([128, HK * DG], F32, tag="bc", name=f"bc{b}")
                nc.scalar.dma_start(bc[:], qdram[b].partition_broadcast(128))
                kt = kts[b]
                # products on GPSIMD (early batches) or DVE; segmented
                # d-reduce on DVE either way.
                pt = pp.tile([128, SCH * HK * DG], F32, tag="pt",
                             name=f"pt{b}")
                if b < pool_nb:
                    for sc in range(SCH):
                        nc.gpsimd.tensor_mul(
                            pt[:, sc * HK * DG:(sc + 1) * HK * DG],
                            kt[:, sc * HK * DG:(sc + 1) * HK * DG],
                            bc[:])
                else:
                    for sc in range(SCH):
                        nc.vector.tensor_mul(
                            pt[:, sc * HK * DG:(sc + 1) * HK * DG],
                            kt[:, sc * HK * DG:(sc + 1) * HK * DG],
                            bc[:])
                nc.vector.tensor_reduce(
                    stall_v[:, :, b, :],
                    pt[:].rearrange("p (sc h d) -> p sc h d", sc=SCH, h=HK),
                    axis=AX.X, op=OP.add)

            # ---- transpose score columns -> rows [npairs, S], r=(b h) ----
            scores = scp.tile([npairs, S], F32, tag="scores")
            for sc in range(SCH):
                sp = tpsp.tile([npairs, 128], F32, tag="tp", name=f"sp{sc}")
                nc.tensor.transpose(sp[:], stall[:, sc * npairs:(sc + 1) * npairs],
                                    eye_sb[:])
                nc.scalar.copy(scores[:, sc * 128:(sc + 1) * 128], sp[:])

            # ---- normalized per-row bisection for 48th-largest ----
            el = scores[:, 0:NSTOP]
            rmax = bp.tile([npairs, 1], F32, tag="rmax")
            nc.vector.tensor_reduce(rmax[:], el, axis=AX.X, op=OP.max)
            rmin = bp.tile([npairs, 1], F32, tag="rmin")
            nc.vector.tensor_reduce(rmin[:], el, axis=AX.X, op=OP.min)
            lo0 = bp.tile([npairs, 1], F32, tag="lo0")
            nc.vector.tensor_scalar_add(lo0[:], rmin[:], -1.0)
            w0 = bp.tile([npairs, 1], F32, tag="w0")
            nc.vector.tensor_sub(w0[:], rmax[:], lo0[:])
            winv = bp.tile([npairs, 1], F32, tag="winv")
            nc.vector.reciprocal(winv[:], w0[:])
            # eln = (el - lo0) * winv in (0, 1]
            eln = scp.tile([npairs, NSTOP], F32, tag="eln")
            nc.vector.tensor_scalar(
                out=eln[:], in0=el, scalar1=lo0[:], scalar2=winv[:],
                op0=OP.subtract, op1=OP.mult)
            ones_w = scp.tile([npairs, NSTOP], F32, tag="ones")
            nc.vector.memset(ones_w[:], 1.0)
            scr = scp.tile([npairs, NSTOP], F32, tag="scr")

            # Invariant: count(> lo) > KEXTRA >= count(> lo + 2^-k); mid = lo + 2^-k.
            mid = bp.tile([npairs, 1], F32, tag="mid")
            nc.vector.memset(mid[:], 0.5)
            cnt = bp.tile([npairs, 1], F32, tag="cnt")
            tt = bp.tile([npairs, 1], F32, tag="tt")
            for it in range(1, n_iter):
                nc.vector.scalar_tensor_tensor(
                    out=scr[:], in0=eln[:], scalar=mid[:], in1=ones_w[:],
                    op0=OP.is_gt, op1=OP.mult, accum_out=cnt[:])
                # tt = (cnt > K) * 2^-it ; mid += tt - 2^-(it+1)
                nc.vector.tensor_scalar(
                    out=tt[:], in0=cnt[:], scalar1=float(KEXTRA),
                    scalar2=float(2.0 ** (-it)), op0=OP.is_gt, op1=OP.mult)
                nc.vector.tensor_scalar(
                    out=mid[:], in0=tt[:], scalar1=float(-(2.0 ** (-(it + 1)))),
                    scalar2=mid[:], op0=OP.add, op1=OP.add)
            # final count at mid_n; thr = mid_n + (cnt>K)*2^-n
            nc.vector.scalar_tensor_tensor(
                out=scr[:], in0=eln[:], scalar=mid[:], in1=ones_w[:],
                op0=OP.is_gt, op1=OP.mult, accum_out=cnt[:])
            thr = bp.tile([npairs, 1], F32, tag="thr")
            nc.vector.tensor_scalar(
                out=thr[:], in0=cnt[:], scalar1=float(KEXTRA),
                scalar2=float(2.0 ** (-n_iter)), op0=OP.is_gt, op1=OP.mult)
            nc.vector.tensor_add(thr[:], thr[:], mid[:])

            # ---- mask assembly: (eln > thr) | sliding ----
            mk = scp.tile([npairs, S], U8, tag="mk")
            nc.vector.scalar_tensor_tensor(
                out=mk[:, 0:NSTOP], in0=eln[:], scalar=thr[:], in1=ones_w[:],
                op0=OP.is_gt, op1=OP.mult)
            nc.vector.memset(mk[:, NSTOP:S], 1)
            nc.scalar.dma_start(mask_u8[:, :], mk[:])

    return nc


def _prep_core_inputs(q, k, wq, cos, sin, c, bl=BL):
    b0, b1 = c * bl, (c + 1) * bl
    # qTg: (bl, HK, G, DM) -> [DM, (h g b)]
    qv = q[b0:b1, 0].reshape(bl, HK, G, DM)
    qTg = np.ascontiguousarray(
        qv.transpose(3, 1, 2, 0).reshape(DM, HK * G * bl))
    # wqg: (HK, G, DM, DG) -> [DM, (h g o)]
    wqg = np.ascontiguousarray(
        wq.transpose(2, 0, 1, 3).reshape(DM, HK * G * DG))
    kcc = np.ascontiguousarray(k[b0:b1])
    # cos8 / sinR8: [bl, HK*DG]; sinR8 folds the rotate-half sign:
    # qdN[d] = qd[d]*cos[d] + qd[swap(d)]*sinR[d], sinR = [-sin[:64], sin[64:]]
    cosb = cos[b0:b1, 0]                      # [bl, DG]
    sinb = sin[b0:b1, 0].copy()
    sinR = sinb.copy()
    sinR[:, :DG // 2] = -sinb[:, :DG // 2]
    cos8t = np.ascontiguousarray(np.tile(cosb, (1, HK)))
    sinR8t = np.ascontiguousarray(np.tile(sinR, (1, HK)))
    return {
        "qTg": qTg, "wqg": wqg, "kc": kcc,
        "cos8": cos8t, "sinR8": sinR8t,
        "eye": np.eye(128, dtype=np.float32),
    }


_CACHE = {}


def kernel(q, k_compressed, wq, cos, sin, attention_mask, block_budget,
           block_sliding_window_size):
    assert int(block_budget) == BUDGET and int(block_sliding_window_size) == SW
    q = np.asarray(q, dtype=np.float32)
    k_compressed = np.asarray(k_compressed, dtype=np.float32)
    wq = np.asarray(wq, dtype=np.float32)
    cos = np.asarray(cos, dtype=np.float32)
    sin = np.asarray(sin, dtype=np.float32)
    attention_mask = np.asarray(attention_mask).astype(bool)

    from concourse import bass_utils

    if "nc" not in _CACHE:
        nc = build_nc()
        if not nc.is_finalized():
            nc.finalize()
        _CACHE["nc"] = nc
    nc = _CACHE["nc"]

    in_maps = [
        _prep_core_inputs(q, k_compressed, wq, cos, sin, c) for c in range(NCORES)
    ]
    res = bass_utils.run_bass_kernel_spmd(nc, in_maps, core_ids=list(range(NCORES)))

    full = np.empty((B, HK, S), dtype=bool)
    for c in range(NCORES):
        m = res.results[c]["mask_u8"].reshape(BL, HK, S).astype(bool)
        full[c * BL:(c + 1) * BL] = m

    full &= attention_mask[:, 0][:, None, :]
    full[:, :, -1] = True
    return full
